# revision 13
# baseline (speedup 1.0000x reference)
"""Trainium2 Bass kernel for nn_Aggregator (GNN message passing).

Computation per (b, e):
  scores[k] = <side[b], rel[b,e,k,:]>          (contract over D=64)
  attn      = softmax_k(scores)
  agg[d]    = sum_k attn[k] * nbr[b,e,k,d]     (contract over K=32)
  out       = relu(cat(self[b,e], agg) @ W + bias)

Sharding: data-parallel over the leading batch dim B=1024 across 8 cores
(128 batches/core); weights replicated.

Per-core design (all data fp16 on the wire; ~18.4 MB DMA/core):
  - partitions = the 128 batches of the core, end to end (no remaps)
  - scores on PE: scores[b,(e,k)] = sum_d diag(side[:,d]) @ rel_d, i.e. 64
    accumulating fp16 matmuls per e-quarter whose stationary weights are
    host-packed diagonal matrices diag(side[:, d]); DVE does no scores work
  - softmax: exp on ACT (PSUM -> SBUF, bf16 for range); row-sums, recip and
    the esc*1/sum normalize on DVE (tiny)
  - agg on DVE: broadcast-mul (fp16 => 2x mode) + log2(K) tree of fp16 adds
    (2x) instead of TensorReduce (which has no fast mode)
  - linear on PE in fp16: per-e transposes of agg land aggT at partition 0
    (PE requires base partition 0 on matmul operands), selfT host-transposed,
    bias as a rank-1 ones x bvec matmul, relu+downcast to fp16 on ACT
  - 4 e-quarters pipeline PE scores -> ACT exp -> DVE agg -> PE linear under
    the DMA stream; inputs DMA in consumption order on the sync queue
"""

import numpy as np

B, E, K, D = 1024, 32, 32, 64
NCORES = 8
BC = B // NCORES   # 128 batches per core
NQ = 4             # pipeline phases
QE = E // NQ       # 8 e's per quarter
GE = 4             # e's per nbr group / DVE tile
NG = QE // GE      # 2 groups per quarter
NC_ = 2            # rel chunks per quarter (32 d each)

_CACHE = {}


def _build_nc():
    from contextlib import ExitStack

    import concourse.bass as bass
    import concourse.bacc as bacc
    import concourse.tile as tile
    from concourse import mybir

    f32 = mybir.dt.float32
    f16 = mybir.dt.float16
    bf16 = mybir.dt.bfloat16
    Alu = mybir.AluOpType
    Act = mybir.ActivationFunctionType

    nc = bacc.Bacc()

    # HBM layouts (host-packed, fp16):
    #   relq[2q+c, b, (ds, e, k)] = rel[b, 8q+e, k, 32c+ds]   ds<32, e<8, k<32
    #   nbrq[2q+g, b, (e4, d, k)] = nbr[b, 8q+4g+e4, k, d]    e4<4, d<64, k<32
    #   diag[p, (d, f)]           = side[p, d] * delta(p, f)
    #   selft[d, (e, b)]          = self[b, e, d]
    #   outh[b, (e, d)]           = out[b, e, d]
    relq_h = nc.declare_dram_parameter("relq", [8, BC, 32 * QE * K], f16, isOutput=False)
    nbrq_h = nc.declare_dram_parameter("nbrq", [8, BC, GE * D * K], f16, isOutput=False)
    side_h = nc.declare_dram_parameter("side", [BC, D], f32, isOutput=False)
    selft_h = nc.declare_dram_parameter("selft", [D, E * BC], f16, isOutput=False)
    wf_h = nc.declare_dram_parameter("wful", [2 * D, D], f16, isOutput=False)
    b_h = nc.declare_dram_parameter("bvec", [1, QE * D], f16, isOutput=False)
    ones_h = nc.declare_dram_parameter("ones", [1, 128], f16, isOutput=False)
    iden_h = nc.declare_dram_parameter("iden", [128, 128], f16, isOutput=False)
    out_h = nc.declare_dram_parameter("outh", [BC, E * D], f16, isOutput=True)

    def vw(t, off_el, dims):
        """View of tile/AP t at extra element offset with given free dims."""
        a = t[:] if hasattr(t, "tile_id") else t
        return bass.AP(tensor=a.tensor, offset=a.offset + off_el, ap=[a.ap[0]] + dims)

    with tile.TileContext(nc) as tc, ExitStack() as ctx:
        consts = ctx.enter_context(tc.tile_pool(name="consts", bufs=1))
        rel_pool = ctx.enter_context(tc.tile_pool(name="rel", bufs=3))
        nbr_pool = ctx.enter_context(tc.tile_pool(name="nbr", bufs=3))
        prod_pool = ctx.enter_context(tc.tile_pool(name="prod", bufs=2))
        tmp_pool = ctx.enter_context(tc.tile_pool(name="tmp", bufs=2))
        work = ctx.enter_context(tc.tile_pool(name="work", bufs=1))
        ps_sc = ctx.enter_context(tc.tile_pool(name="ps_sc", bufs=4, space="PSUM"))
        ps_tr = ctx.enter_context(tc.tile_pool(name="ps_tr", bufs=2, space="PSUM"))
        ps_lin = ctx.enter_context(tc.tile_pool(name="ps_lin", bufs=2, space="PSUM"))

        # ---- constants ----
        wf_sb = consts.tile([2 * D, D], f16)
        nc.sync.dma_start(out=wf_sb, in_=wf_h[:])
        bvec_sb = consts.tile([1, QE * D], f16)
        nc.sync.dma_start(out=bvec_sb, in_=b_h[:])
        ones_sb = consts.tile([1, 128], f16)
        nc.sync.dma_start(out=ones_sb, in_=ones_h[:])
        iden_sb = consts.tile([128, 128], f16)
        nc.sync.dma_start(out=iden_sb, in_=iden_h[:])
        # xt[0:64, e*128:+128] = selfT_e ; rows 64:128 get aggT_e per quarter
        xt = consts.tile([2 * D, E * BC], f16)
        nc.sync.dma_start(out=xt[0:D], in_=selft_h[:])
        side_sb = consts.tile([BC, D], f32)
        nc.sync.dma_start(out=side_sb, in_=side_h[:])
        # build diag(side[:, d]) on-chip: diag_d = iden * side[:, d] (TSP, 4x
        # mode, ~0.1us each) during DVE's otherwise-idle startup window
        diag_sb = consts.tile([BC, D * 128], f16)
        for d in range(D):
            nc.vector.tensor_scalar_mul(
                out=vw(diag_sb, d * 128, [[1, 128]]),
                in0=iden_sb,
                scalar1=side_sb[:, d : d + 1],
            )

        # ---- streamed inputs: one sync queue, consumption order ----
        rel_t = {}
        nbr_t = {}
        for q in range(NQ):
            for c in range(NC_):
                t = rel_pool.tile([BC, 32 * QE * K], f16, tag="rel")
                nc.sync.dma_start(out=t, in_=relq_h[:][NC_ * q + c])
                rel_t[q, c] = t
            for g in range(NG):
                t = nbr_pool.tile([BC, GE * D * K], f16, tag="nbr")
                nc.sync.dma_start(out=t, in_=nbrq_h[:][NG * q + g])
                nbr_t[q, g] = t

        out_all = work.tile([BC, E * D], f16)

        # ---- phase A: all scores chains back-to-back on PE, exps on ACT ----
        esc_q = {}
        for q in range(NQ):
            sc_ps = ps_sc.tile([BC, QE * K], f32, tag="sc")
            for c in range(NC_):
                rt = rel_t[q, c]
                for ds in range(32):
                    d = 32 * c + ds
                    nc.tensor.matmul(
                        out=sc_ps,
                        lhsT=vw(diag_sb, d * 128, [[1, 128]]),
                        rhs=vw(rt, ds * QE * K, [[1, QE * K]]),
                        start=(d == 0),
                        stop=(d == 63),
                    )
            esc = work.tile([BC, QE * K], bf16, tag=f"esc{q}")
            nc.scalar.activation(out=esc, in_=sc_ps, func=Act.Exp)
            esc_q[q] = esc

        # ---- phase B: per-quarter softmax smalls + agg on DVE ----
        agg_q = {}
        for q in range(NQ):
            esc = esc_q[q]
            sums = work.tile([BC, QE], f32, tag=f"sums{q}")
            nc.vector.tensor_reduce(
                out=sums,
                in_=vw(esc, 0, [[K, QE], [1, K]]),
                axis=mybir.AxisListType.X,
                op=Alu.add,
            )
            rs = work.tile([BC, QE], f32, tag=f"rs{q}")
            nc.vector.reciprocal(out=rs, in_=sums)
            esc_n = work.tile([BC, QE * K], f16, tag=f"escn{q}")
            nc.vector.tensor_mul(
                out=vw(esc_n, 0, [[K, QE], [1, K]]),
                in0=vw(esc, 0, [[K, QE], [1, K]]),
                in1=vw(rs, 0, [[1, QE], [0, K]]),
            )
            agg = work.tile([BC, QE * D], f16, tag=f"agg{q}")
            for g in range(NG):
                p2 = prod_pool.tile([BC, GE * D * K], f16, tag="p2")
                nc.vector.tensor_mul(
                    out=vw(p2, 0, [[D * K, GE], [K, D], [1, K]]),
                    in0=vw(nbr_t[q, g], 0, [[D * K, GE], [K, D], [1, K]]),
                    in1=vw(esc_n, g * GE * K, [[K, GE], [0, D], [1, K]]),
                )
                src, kk = p2, K
                while kk > 2:
                    kk //= 2
                    dst = tmp_pool.tile([BC, GE * D * kk], f16, tag="tr")
                    nc.vector.tensor_add(
                        out=vw(dst, 0, [[kk, GE * D], [1, kk]]),
                        in0=vw(src, 0, [[2 * kk, GE * D], [1, kk]]),
                        in1=vw(src, kk, [[2 * kk, GE * D], [1, kk]]),
                    )
                    src = dst
                nc.vector.tensor_add(
                    out=vw(agg, g * GE * D, [[1, GE * D]]),
                    in0=vw(src, 0, [[2, GE * D]]),
                    in1=vw(src, 1, [[2, GE * D]]),
                )
            agg_q[q] = agg

        # ---- phase C: per-quarter aggT transposes (2 e's each) + linear ----
        for q in range(NQ):
            agg = agg_q[q]
            for c in range(QE // 2):
                tp = ps_tr.tile([128, 128], f16, tag="tp")
                nc.tensor.transpose(
                    out=tp, in_=vw(agg, c * 128, [[1, 128]]), identity=iden_sb
                )
                e0 = QE * q + 2 * c
                nc.scalar.copy(out=xt[D : 2 * D, e0 * BC : (e0 + 1) * BC], in_=tp[0:D])
                nc.scalar.copy(
                    out=xt[D : 2 * D, (e0 + 1) * BC : (e0 + 2) * BC], in_=tp[D : 2 * D]
                )
            lin = ps_lin.tile([BC, QE * D], f32, tag="lin")
            nc.tensor.matmul(out=lin, lhsT=ones_sb, rhs=bvec_sb, start=True, stop=True)
            for i in range(QE):
                e = QE * q + i
                nc.tensor.matmul(
                    out=vw(lin, i * D, [[1, D]]),
                    lhsT=xt[:, e * BC : (e + 1) * BC],
                    rhs=wf_sb,
                    start=False,
                    stop=True,
                    skip_group_check=True,
                )
            nc.scalar.activation(
                out=vw(out_all, q * QE * D, [[1, QE * D]]),
                in_=lin,
                func=Act.Relu,
            )

        # ---- output DMAs (per quarter, fire as relus complete) ----
        for q in range(NQ):
            nc.sync.dma_start(
                out=vw(out_h[:], q * QE * D, [[1, QE * D]]),
                in_=vw(out_all, q * QE * D, [[1, QE * D]]),
            )

    nc.finalize()
    return nc


def _get_nc():
    if "nc" not in _CACHE:
        _CACHE["nc"] = _build_nc()
    return _CACHE["nc"]


def _make_in_maps(self_vectors, neighbor_vectors, neighbor_relations, side_embeddings, W, b):
    f16 = np.float16
    iden = np.eye(128, dtype=f16)
    ones = np.ones((1, 128), dtype=f16)
    wful = np.ascontiguousarray(np.asarray(W, dtype=f16))
    bvec = np.ascontiguousarray(np.tile(np.asarray(b, dtype=f16), QE)).reshape(1, QE * D)
    rel = np.asarray(neighbor_relations, dtype=f16)
    nbr = np.asarray(neighbor_vectors, dtype=f16)
    sv = np.asarray(self_vectors, dtype=f16)
    side = np.asarray(side_embeddings, dtype=np.float32)
    in_maps = []
    for c in range(NCORES):
        sl = slice(c * BC, (c + 1) * BC)
        # relq[2q+cc, b, ds, e, k] = rel[b, 8q+e, k, 32cc+ds]
        r = rel[sl].reshape(BC, NQ, QE, K, NC_, 32)    # b q e k cc ds
        r = r.transpose(1, 4, 0, 5, 2, 3)              # q cc b ds e k
        relq = np.ascontiguousarray(r).reshape(8, BC, 32 * QE * K)
        # nbrq[2q+g, b, e4, d, k] = nbr[b, 8q+4g+e4, k, d]
        n = nbr[sl].reshape(BC, NQ * NG, GE, K, D)     # b qg e4 k d
        n = n.transpose(1, 0, 2, 4, 3)                 # qg b e4 d k
        nbrq = np.ascontiguousarray(n).reshape(8, BC, GE * D * K)
        selft = np.ascontiguousarray(sv[sl].transpose(2, 1, 0)).reshape(D, E * BC)
        in_maps.append(
            {
                "relq": relq,
                "nbrq": nbrq,
                "side": np.ascontiguousarray(side[sl], dtype=np.float32),
                "selft": selft,
                "wful": wful,
                "bvec": bvec,
                "ones": ones,
                "iden": iden,
            }
        )
    return in_maps


def kernel(self_vectors, neighbor_vectors, neighbor_relations, side_embeddings, W, b,
           _trace=False, _tmpdir=None):
    from concourse import bass_utils

    nc = _get_nc()
    in_maps = _make_in_maps(
        self_vectors, neighbor_vectors, neighbor_relations, side_embeddings, W, b
    )
    res = bass_utils.run_bass_kernel_spmd(
        nc, in_maps, list(range(NCORES)), trace=_trace, tmpdir=_tmpdir
    )
    _CACHE["last_results"] = res
    out = np.concatenate(
        [
            res.results[c]["outh"].astype(np.float32).reshape(BC, E, D)
            for c in range(NCORES)
        ],
        axis=0,
    )
    return out
